# revision 65
# baseline (speedup 1.0000x reference)
"""DenseGAT layer on 8 Trainium2 NeuronCores (row-parallel over nodes).

Math (reference):
    A = adj + I; mask = A > 0
    Wh = h @ W.T;  el = Wh @ a_left;  er = Wh @ a_right
    e_ij = LeakyReLU_0.2(el_i + er_j); masked softmax over j; att = softmax * A
    out = att @ Wh

Key identity used here: exp(lrelu(x)) = max(exp(x), exp(0.2 x)) and
exp(el_i + er_j) = exp(el_i) exp(er_j), so with P=exp(el), Q=exp(er),
R=exp(.2 el), S=exp(.2 er):
    q_ij   = exp(lrelu(e_ij)) = max(P_i Q_j, R_i S_j)     (rank-1 products)
    num_i  = sum_j q_ij A_ij Wh_j                          (PE matmul)
    den_i  = sum_j q_ij [A_ij > 0]                         (fused DVE accum)
    out_i  = num_i / den_i
Softmax without max-subtraction is exact here (|e| <= ~10, exp stays finite).

Sharding: core c owns rows [c*1024, (c+1)*1024). Columns (and h/Wh rows,
consistently) are rotated per-core so the core's own 1024 columns come first;
that puts the A+I diagonal blocks at compile-time-constant local positions,
keeping the program SPMD. Output rows are unpermuted, so the host just
concatenates the 8 row blocks.
"""

import numpy as np
import ml_dtypes

import concourse.bass as bass
import concourse.mybir as mybir
import concourse.tile as tile
from concourse.bass_utils import run_bass_kernel_spmd

N = 8192
F_IN = 128
F_OUT = 64
NCORES = 8
RPC = N // NCORES          # rows per core (1024)
NT = RPC // 128            # row tiles per core (8)
JT = N // 128              # column tiles (64)
CH = 2048                  # steady-state free-dim chunk
NCH = N // CH              # chunks per row tile (4)
LEAKY = 0.2

F32 = mybir.dt.float32
BF16 = mybir.dt.bfloat16
AX = mybir.AxisListType
OP = mybir.AluOpType
AF = mybir.ActivationFunctionType


MAX_WAITS_PER_INST = 1


def _spill_excess_waits(nc):
    # Walrus CoreV3 codegen rejects instructions carrying more than
    # MAX_WAITS_PER_INST sem waits, but Tile's sem assignment can attach one
    # wait per producer lane. Spill the extras onto nop instructions spliced
    # immediately before the instruction in the same engine stream — the HW
    # executes each engine's instructions in block order, so waiting on a
    # preceding same-engine nop is equivalent.
    spill_idx = 0
    for f in nc.m.functions:
        for bb in f.blocks:
            out = []
            for inst in bb.instructions:
                si = inst.sync_info
                if si is not None and si.on_wait and len(si.on_wait) > MAX_WAITS_PER_INST:
                    waits = list(si.on_wait)
                    extra, keep = (
                        waits[: -MAX_WAITS_PER_INST],
                        waits[-MAX_WAITS_PER_INST:],
                    )
                    while extra:
                        chunk, extra = extra[:MAX_WAITS_PER_INST], extra[MAX_WAITS_PER_INST:]
                        spill_idx += 1
                        n = mybir.InstNoOp(
                            name=f"I-waitspill-{spill_idx}",
                            hint="wait_spill",
                            engine=inst.engine,
                        )
                        n.sync_info = mybir.SyncInfo(on_wait=chunk, on_update=[])
                        nc.register_instruction(n)
                        out.append(n)
                    si.on_wait = keep
                out.append(inst)
            bb.instructions[:] = out


def build_module(skip=()):
    # `skip` (ablation, perf analysis only): subset of
    # {"mm2", "trans", "copies", "dve", "dma"} — omit that stage.
    nc = bass.Bass("TRN2", target_bir_lowering=False, debug=False)

    adj_s = nc.dram_tensor("adj_s", [RPC, N], F32, kind="ExternalInput").ap()
    hT = nc.dram_tensor("hT", [F_IN, N], F32, kind="ExternalInput").ap()
    wT = nc.dram_tensor("wT", [F_IN, F_OUT], F32, kind="ExternalInput").ap()
    W_ = nc.dram_tensor("W_", [F_OUT, F_IN], F32, kind="ExternalInput").ap()
    ar = nc.dram_tensor("ar", [F_OUT, 1], F32, kind="ExternalInput").ap()
    al = nc.dram_tensor("al", [F_OUT, 1], F32, kind="ExternalInput").ap()
    AL = nc.dram_tensor("AL", [128, F_OUT], F32, kind="ExternalInput").ap()
    ident = nc.dram_tensor("ident", [128, 128], BF16, kind="ExternalInput").ap()
    id64 = nc.dram_tensor("id64", [F_OUT, F_OUT], F32, kind="ExternalInput").ap()
    ones1 = nc.dram_tensor("ones1", [1, 128], F32, kind="ExternalInput").ap()
    out_d = nc.dram_tensor("out", [RPC, F_OUT], F32, kind="ExternalOutput").ap()

    with tile.TileContext(nc) as tc:
        with (
            tc.tile_pool(name="persist", bufs=1) as persist,
            tc.tile_pool(name="consts", bufs=1) as consts,
            tc.tile_pool(name="apool", bufs=6) as apool,
        ):
            # ---- constant loads -------------------------------------------
            # (emission order = DMA priority: the er-chain constants and hT
            # chunk 0 gate the whole steady state, so they go first)
            W_sb = consts.tile([F_OUT, F_IN], F32, tag="W_")
            nc.sync.dma_start(W_sb[:], W_[:])
            ar_sb = consts.tile([F_OUT, 1], F32, tag="ar")
            nc.sync.dma_start(ar_sb[:], ar[:])
            al_sb = consts.tile([F_OUT, 1], F32, tag="al")
            nc.sync.dma_start(al_sb[:], al[:])
            ones_sb = consts.tile([1, 128], F32, tag="ones1")
            nc.sync.dma_start(ones_sb[:], ones1[:])

            # ---- persistent products --------------------------------------
            # Chunked into separate tiles: Tile dependencies are
            # tile-granular, so a consumer of one chunk must not be chained
            # to every producer of a monolithic tensor.
            QQs = [persist.tile([128, CH], BF16, name=f"QQ{c}", tag=f"QQ{c}") for c in range(NCH)]
            SSs = [persist.tile([128, CH], BF16, name=f"SS{c}", tag=f"SS{c}") for c in range(NCH)]
            Whs = [persist.tile([128, 8 * F_OUT], BF16, name=f"Wh{g}", tag=f"Wh{g}")
                   for g in range(JT // 8)]
            el_all = persist.tile([128, NT], F32, tag="el")
            P_all = persist.tile([128, NT], F32, tag="P")
            R_all = persist.tile([128, NT], F32, tag="R")
            den_parts = persist.tile([128, NT * NCH], F32, tag="denp")
            den_all = persist.tile([128, NT], F32, tag="den")
            rden_all = persist.tile([128, NT], F32, tag="rden")

            # A loads are prefetched during the prologue; SWDGE runs on the
            # otherwise-idle Pool queue and the DMA engines overlap prologue
            # compute.
            a_tiles = {}

            def load_A(it):
                halves = []
                for h2 in range(2):
                    at = apool.tile(
                        [128, N // 2], BF16, name=f"A_{it}_{h2}", tag="A"
                    )
                    if "dma" not in skip:
                        nc.gpsimd.dma_start(
                            at[:],
                            adj_s[
                                it * 128 : (it + 1) * 128,
                                h2 * (N // 2) : (h2 + 1) * (N // 2),
                            ],
                        )
                    halves.append(at)
                a_tiles[it] = halves

            # Steady-state SBUF pools open BEFORE the prologue scratch pool:
            # the stack allocator would otherwise hand the steady tiles the
            # prologue's addresses, chaining the first steady op to the last
            # prologue consumer.
            dvep = tc.alloc_tile_pool(name="dvep", bufs=3)
            gtp = tc.alloc_tile_pool(name="gtp", bufs=3)
            tailp = tc.alloc_tile_pool(name="tailp", bufs=2)

            # ---- prologue: er -> QQ/SS (critical path), then Wh, el -------
            with (
                tc.tile_pool(name="pscr", bufs=2) as pscr,
                tc.tile_pool(name="psW", bufs=2, space="PSUM") as psW,
                tc.tile_pool(name="psT", bufs=2, space="PSUM") as psT,
                tc.tile_pool(name="psB", bufs=2, space="PSUM") as psB,
            ):
                hTs = []
                for hc in range(NCH):
                    t = pscr.tile([F_IN, CH], F32, tag=f"hT{hc}", bufs=1)
                    nc.sync.dma_start(
                        t[:], hT[:, hc * CH : (hc + 1) * CH]
                    )
                    hTs.append(t)
                # remaining constants (needed later than the er chain)
                wT_sb = consts.tile([F_IN, F_OUT], F32, tag="wT")
                nc.sync.dma_start(wT_sb[:], wT[:])
                AL_sb = consts.tile([128, F_OUT], F32, tag="AL")
                nc.sync.dma_start(AL_sb[:], AL[:])
                id_sb = consts.tile([128, 128], BF16, tag="ident")
                nc.sync.dma_start(id_sb[:], ident[:])
                id64_sb = consts.tile([F_OUT, F_OUT], F32, tag="id64")
                nc.sync.dma_start(id64_sb[:], id64[:])

                # Attention projections without materializing Wh first:
                # er = Wh@ar = h@(W.T@ar) = hT.T-contraction with war, and
                # el likewise with wal — one tiny matmul each, then the big
                # products come straight off the hT chunks.
                pv = psW.tile([F_IN, 2], F32, tag="pv", bufs=1)
                nc.tensor.matmul(pv[:, 0:1], lhsT=W_sb[:], rhs=ar_sb[:],
                                 start=True, stop=True)
                nc.tensor.matmul(pv[:, 1:2], lhsT=W_sb[:], rhs=al_sb[:],
                                 start=True, stop=True)
                wv_sb = pscr.tile([F_IN, 2], F32, tag="wv", bufs=1)
                nc.vector.tensor_copy(wv_sb[:], pv[:])

                # el per row-tile (core rows = hT chunk 0 columns): [128,1]
                pel = psW.tile([128, NT], F32, tag="pel", bufs=1)
                for it in range(NT):
                    nc.tensor.matmul(
                        pel[:, it : it + 1],
                        lhsT=hTs[0][:, it * 128 : (it + 1) * 128],
                        rhs=wv_sb[:, 1:2],
                        start=True,
                        stop=True,
                    )
                nc.vector.tensor_copy(el_all[:], pel[:])
                nc.scalar.activation(P_all[:], el_all[:], AF.Exp)
                nc.scalar.activation(R_all[:], el_all[:], AF.Exp, scale=LEAKY)

                # er chain per 512-chunk: er = war.T @ hT -> PE
                # ones-broadcast -> exp straight out of PSUM. QQ chunk cb is
                # ready as soon as its own four sub-chains complete.
                for cb in range(NCH):
                    for k in range(CH // 512):
                        sl512 = slice(k * 512, (k + 1) * 512)
                        per_ = psT.tile([1, 512], F32, tag="per")
                        nc.tensor.matmul(
                            per_[:],
                            lhsT=wv_sb[:, 0:1],
                            rhs=hTs[cb][:, sl512],
                            start=True,
                            stop=True,
                        )
                        er_k = pscr.tile([1, 512], F32, tag="erk")
                        nc.scalar.activation(er_k[:], per_[:], AF.Copy)
                        # replicate er across partitions: ones^T @ er
                        pbc = psB.tile([128, 512], F32, tag="pbc")
                        nc.tensor.matmul(
                            pbc[:], lhsT=ones_sb[:], rhs=er_k[:],
                            start=True, stop=True,
                        )
                        nc.scalar.activation(
                            QQs[cb][:, sl512], pbc[:], AF.Exp
                        )
                        nc.scalar.activation(
                            SSs[cb][:, sl512], pbc[:], AF.Exp, scale=LEAKY
                        )

                load_A(0)
                load_A(1)

                # Wh j-tiles (needed only by the num matmuls)
                for g in range(JT // 8):
                    pwh = psW.tile([128, 8 * F_OUT], F32, tag="pwh")
                    for k in range(8):
                        jt = g * 8 + k
                        nc.tensor.matmul(
                            pwh[:, k * F_OUT : (k + 1) * F_OUT],
                            lhsT=hTs[(jt * 128) // CH][
                                :, (jt * 128) % CH : (jt * 128) % CH + 128
                            ],
                            rhs=wT_sb[:],
                            start=True,
                            stop=True,
                        )
                    nc.scalar.activation(Whs[g][:], pwh[:], AF.Copy)

            # ---- steady state ---------------------------------------------
            with (
                tc.tile_pool(name="psGT", bufs=3, space="PSUM") as psGT,
                tc.tile_pool(name="psN", bufs=1, space="PSUM") as psN,
                tc.tile_pool(name="psO", bufs=1, space="PSUM") as psO,
            ):
                for it in range(NT):
                    # keep 2 row-tiles of A in flight ahead of the consumer
                    for pre in (it, it + 1, it + 2):
                        if pre < NT and pre not in a_tiles:
                            load_A(pre)
                    halves = a_tiles.pop(it)
                    # self-loops: diagonal block is local block `it`
                    dsl = slice(it * 128, (it + 1) * 128)
                    if "dve" not in skip:
                        # on Pool: keeps the A-DMA wait off the DVE queue
                        # head (the score ops u/v/max don't need A)
                        nc.gpsimd.tensor_add(
                            halves[0][:, dsl], halves[0][:, dsl], id_sb[:]
                        )

                    P_col = P_all[:, it : it + 1]
                    R_col = R_all[:, it : it + 1]
                    GTs = [gtp.tile([128, CH], BF16, name=f"GTt{c}", tag=f"GT{c}") for c in range(NCH)]
                    # num^T accumulator for this row-tile: [64, 128] over 64 j-tiles
                    pnum = psN.tile([F_OUT, 128], F32, tag="pnum")

                    for c in range(NCH):
                        Asub = halves[c // (NCH // 2)][
                            :, (c % (NCH // 2)) * CH : (c % (NCH // 2) + 1) * CH
                        ]
                        v = dvep.tile([128, CH], BF16, tag="v")
                        u = dvep.tile([128, CH], BF16, tag="u")
                        if "dve" not in skip:
                            # q = max(P*QQ, R*SS): tensor_scalar runs 4x bf16,
                            # tensor_tensor 2x — cheaper than the 1x fused stt.
                            # q overwrites u; the masked copy overwrites v;
                            # G = q*A overwrites the A chunk (all dead after).
                            nc.vector.tensor_scalar_mul(v[:], SSs[c][:], R_col)
                            nc.vector.tensor_scalar_mul(u[:], QQs[c][:], P_col)
                            nc.vector.tensor_max(u[:], u[:], v[:])
                            # masked q + free row-sum accumulator (denominator);
                            # stt is 1x but fuses mask-gen + mult + reduction.
                            nc.vector.scalar_tensor_tensor(
                                out=v[:],
                                in0=Asub,
                                scalar=0.0,
                                in1=u[:],
                                op0=OP.is_gt,
                                op1=OP.mult,
                                accum_out=den_parts[
                                    :, it * NCH + c : it * NCH + c + 1
                                ],
                            )
                            # numerator weights: G = q * A (in place on A).
                            # Pool takes a slice to offload DVE, but not so
                            # much that its (in-order) queue delays the A-tile
                            # SWDGE descriptor generation.
                            if c in (0, 1):
                                nc.gpsimd.tensor_mul(Asub, u[:], Asub)
                            else:
                                nc.vector.tensor_mul(Asub, u[:], Asub)

                        # transpose G 128x128 blocks on PE (lhsT=G, rhs=I)
                        for g8 in range(CH // 1024):
                            pgt = psGT.tile([128, 1024], F32, tag="pgt")
                            if "trans" not in skip:
                                for k in range(8):
                                    kk = g8 * 8 + k
                                    nc.tensor.matmul(
                                        pgt[:, k * 128 : (k + 1) * 128],
                                        lhsT=Asub[:, kk * 128 : (kk + 1) * 128],
                                        rhs=id_sb[:],
                                        start=True,
                                        stop=True,
                                    )
                            base = g8 * 1024
                            if "copies" not in skip:
                                nc.scalar.activation(
                                    GTs[c][:, base : base + 1024], pgt[:], AF.Copy
                                )

                        # num^T accumulation for this chunk's 16 j-tiles
                        # (emitted per chunk so the psum accumulation trails
                        # the GT copies instead of waiting for the whole
                        # row-tile)
                        if "mm2" not in skip:
                            for kk in range(CH // 128):
                                jt = c * (CH // 128) + kk
                                nc.tensor.matmul(
                                    pnum[:],
                                    lhsT=Whs[jt // 8][
                                        :, (jt % 8) * F_OUT : (jt % 8 + 1) * F_OUT
                                    ],
                                    rhs=GTs[c][:, kk * 128 : (kk + 1) * 128],
                                    start=(jt == 0),
                                    stop=(jt == JT - 1),
                                )

                    if "mm2" not in skip:
                        # tail: denominator, reciprocal, transpose, scale, store
                        nc.vector.tensor_reduce(
                            den_all[:, it : it + 1],
                            den_parts[:, it * NCH : (it + 1) * NCH],
                            axis=AX.X,
                            op=OP.add,
                        )
                        nc.vector.reciprocal(
                            rden_all[:, it : it + 1], den_all[:, it : it + 1]
                        )
                        num_sb = tailp.tile([F_OUT, 128], F32, tag="num")
                        nc.scalar.activation(num_sb[:], pnum[:], AF.Copy)
                        pout = psO.tile([128, F_OUT], F32, tag="pout")
                        nc.tensor.matmul(
                            pout[:],
                            lhsT=num_sb[:],
                            rhs=id64_sb[:],
                            start=True,
                            stop=True,
                        )
                        out_sb = tailp.tile([128, F_OUT], F32, tag="out")
                        nc.scalar.activation(
                            out_sb[:], pout[:], AF.Copy,
                            scale=rden_all[:, it : it + 1],
                        )
                        nc.sync.dma_start(
                            out_d[it * 128 : (it + 1) * 128, :], out_sb[:]
                        )

            tailp.release()
            gtp.release()
            dvep.release()

    _spill_excess_waits(nc)
    return nc


_NC_CACHE = None


def _get_module():
    global _NC_CACHE
    if _NC_CACHE is None:
        _NC_CACHE = build_module()
    return _NC_CACHE


def _prep_inputs(h, W, a_left, a_right, adj):
    h = np.asarray(h, np.float32)
    W = np.asarray(W, np.float32)
    a_left = np.asarray(a_left, np.float32)
    a_right = np.asarray(a_right, np.float32)
    adj = np.asarray(adj, np.float32)

    hT_full = np.ascontiguousarray(h.T)                      # [128, 8192]
    wT_np = np.ascontiguousarray(W.T)                        # [128, 64]
    W_np = np.ascontiguousarray(W)                           # [64, 128]
    ar_np = np.ascontiguousarray(a_right.reshape(F_OUT, 1))
    al_np = np.ascontiguousarray(a_left.reshape(F_OUT, 1))
    AL_np = np.ascontiguousarray(np.broadcast_to(a_left[None, :], (128, F_OUT)))
    id_np = np.eye(128, dtype=ml_dtypes.bfloat16)
    id64_np = np.eye(F_OUT, dtype=np.float32)
    ones_np = np.ones((1, 128), dtype=np.float32)

    in_maps = []
    for c in range(NCORES):
        r0 = c * RPC
        slab = adj[r0 : r0 + RPC]
        # rotate columns: core's own columns first (keeps diagonal blocks at
        # compile-time-constant local positions; output is permutation-invariant)
        adj_c = np.concatenate(
            [slab[:, r0 : r0 + RPC], slab[:, :r0], slab[:, r0 + RPC :]], axis=1
        )
        hT_c = np.concatenate(
            [hT_full[:, r0 : r0 + RPC], hT_full[:, :r0], hT_full[:, r0 + RPC :]],
            axis=1,
        )
        in_maps.append(
            {
                "adj_s": np.ascontiguousarray(adj_c),
                "hT": np.ascontiguousarray(hT_c),
                "wT": wT_np,
                "W_": W_np,
                "ar": ar_np,
                "al": al_np,
                "AL": AL_np,
                "ident": id_np,
                "id64": id64_np,
                "ones1": ones_np,
            }
        )
    return in_maps


def run(inputs: dict, trace: bool = False):
    """Run on 8 cores; returns (output [8192, 64] f32, BassKernelResults)."""
    nc = _get_module()
    in_maps = _prep_inputs(**inputs)
    res = run_bass_kernel_spmd(nc, in_maps, core_ids=list(range(NCORES)), trace=trace)
    out = np.concatenate([res.results[c]["out"] for c in range(NCORES)], axis=0)
    return out, res


def kernel(**inputs) -> np.ndarray:
    out, _ = run(inputs)
    return out
